# revision 9
# baseline (speedup 1.0000x reference)
"""Trainium2 Bass kernel for nn_MultiHeadGraphAttention.

Strategy: data-parallel over batch B=8 -> one batch element per NeuronCore,
zero collectives.  Per core, full 16-head attention over T=1024, D=1024.

v2: software-pipelined schedule.  The attention inner loop is ScalarE-bound
(16.8M exps ~ 110us at 1 elem/cycle/lane) while the projections are PE-bound
(~110us); v1 ran them in serial phases so each engine idled half the time.
v2 interleaves them:

  prologue:  V-projection (all 8 token tiles), expB = exp((sp+ed)^T),
             Q/K projection for head-pair group 0.
  step g=0..7, inner j=0..7:
     - scores for pair g, key-tile j: the two heads' K=64 matmuls sit at
       partition offsets 0/64 so bass auto-derives tile_position
       (0,0)/(64,0) and they run CONCURRENTLY in separate 64-row PE tiles;
       emission order T0,T8,T0,T8 maximizes the overlap.
     - exp + expB-multiply for both heads (ScalarE is the ~2us/j pacer).
     - AV for pair g-1: head h2a on j=0..3, h2b on j=4..7, two key-tiles
       per j, accumulating [65,512] halves (ones-column denominator trick).
     - 4 matmuls of next pair's Q (j<4) / K (j>=4) projection.
  epilogue:  AV(pair 7), output projection.

PSUM: scores 2x[128,1024] (4 banks) + AV 2x[65,512] (2) + proj [128,1024]
(2) = 8 banks exactly.

Other tricks unchanged from v1: fp16 activations/weights, exp(qk+bias) =
exp(qk)*expB with the key-padding mask riding ScalarE's per-partition bias
port, reciprocal_approx_fast on SBUF only, GpSimd partition-broadcast for
the denominator, odd heads staged through SBUF and DMA-copied into the
upper partition half of packed OT tiles.
"""

import os
import sys
from contextlib import ExitStack

import numpy as np

for _p in ("/opt/trn_rl_repo", "/root/.axon_site/_ro/trn_rl_repo"):
    if os.path.isdir(_p) and _p not in sys.path:
        sys.path.insert(0, _p)

import ml_dtypes

B, T, D, H = 8, 1024, 1024, 16
DH = D // H  # 64
P = 128
NT = T // P  # 8 token tiles
ND = D // P  # 8 feature tiles
NP = H // 2  # 8 head pairs
NCORES = 8
SCALE = 1.0 / np.sqrt(np.float32(DH))  # 0.125
NEG = -1.0e9
BF16 = ml_dtypes.bfloat16
FP16 = np.float16

_PROG_CACHE = {}


def build_program(
    bv_nz: bool = False,
    bo_nz: bool = False,
    loop_n: int = 0,
):
    import concourse.bass as bass
    import concourse.tile as tile
    from concourse import bacc, mybir
    from concourse.alu_op_type import AluOpType

    fp32 = mybir.dt.float32
    bf16 = mybir.dt.bfloat16
    fp16 = mybir.dt.float16
    AF = mybir.ActivationFunctionType

    nc = bacc.Bacc("TRN2", target_bir_lowering=False, debug=False)

    def din(name, shape, dt=fp16):
        return nc.dram_tensor(name, shape, dt, kind="ExternalInput").ap()

    xqT_d = din("xqT", [D, T])
    xkT_d = din("xkT", [D, T])
    xvT_d = din("xvT", [D, T])
    spT_d = din("spT", [T, T])
    edT_d = din("edT", [T, T])
    wq_d = din("wq", [D, D])
    wk_d = din("wk", [D, D])
    wv_d = din("wv", [D, D])
    wo_d = din("wo", [D, D])
    maskneg_d = din("maskneg", [P, NT], fp32)
    bq_d = din("bq2", [P, ND], fp32)
    bk_d = din("bk2", [P, ND], fp32)
    bv_d = din("bv1", [1, D], fp32) if bv_nz else None
    bo_d = din("bo1", [1, D], fp32) if bo_nz else None
    y_d = nc.dram_tensor("y", [T, D], fp32, kind="ExternalOutput").ap()

    with tile.TileContext(nc) as tc, ExitStack() as ctx:
        pers = ctx.enter_context(tc.tile_pool(name="pers", bufs=1))
        xpool = ctx.enter_context(tc.tile_pool(name="xp", bufs=8))
        wpool = ctx.enter_context(tc.tile_pool(name="wp", bufs=8))
        epool = ctx.enter_context(tc.tile_pool(name="ep", bufs=16))
        ypool = ctx.enter_context(tc.tile_pool(name="yp", bufs=1))
        spool = ctx.enter_context(tc.tile_pool(name="sp", bufs=2))
        # PSUM budget (8 banks): scores 2x[128,1024]f32 = 4, AV 2x[65,512] = 2,
        # projections 1x[128,1024] = 2.
        ps_sc = ctx.enter_context(
            tc.tile_pool(name="psS", bufs=2, space=bass.MemorySpace.PSUM)
        )
        ps_av = ctx.enter_context(
            tc.tile_pool(name="psA", bufs=2, space=bass.MemorySpace.PSUM)
        )
        ps_pj = ctx.enter_context(
            tc.tile_pool(name="psP", bufs=1, space=bass.MemorySpace.PSUM)
        )
        if loop_n:
            ctx.enter_context(tc.For_i(0, loop_n, 1))

        # ---- small constants ----
        maskb_t = pers.tile([P, NT], fp32, tag="maskb")
        nc.sync.dma_start(maskb_t, maskneg_d)
        bq_t = pers.tile([P, ND], fp32, tag="bqt")
        nc.sync.dma_start(bq_t, bq_d)
        bk_t = pers.tile([P, ND], fp32, tag="bkt")
        nc.sync.dma_start(bk_t, bk_d)
        bvb_t = None
        if bv_nz:
            bv_row = pers.tile([1, D], fp32, tag="bvrow")
            nc.sync.dma_start(bv_row, bv_d)
            bvb_t = pers.tile([P, D], fp32, tag="bvb")
            nc.gpsimd.partition_broadcast(bvb_t, bv_row)
        bob_t = None
        if bo_nz:
            bo_row = pers.tile([1, D], fp32, tag="borow")
            nc.sync.dma_start(bo_row, bo_d)
            bob_t = pers.tile([P, D], fp32, tag="bob")
            nc.gpsimd.partition_broadcast(bob_t, bo_row)

        # ---- persistent activation tensors ----
        # qT/kT group tiles live only from their projection (step g-1) to
        # their scores (step g): ring of 3 instead of 8 persistent tiles.
        qT_t = {}
        kT_t = {}
        # V: per token-tile, heads interleaved in 65-column blocks (65th = 1.0)
        v_t = [pers.tile([P, H * (DH + 1)], bf16, tag=f"v{i}", name=f"v{i}") for i in range(NT)]
        oT_t = [pers.tile([P, T], fp16, tag=f"oT{i}", name=f"oT{i}") for i in range(ND)]
        expB_t = [
            pers.tile([P, T], bf16, tag=f"expB{j}", name=f"expB{j}") for j in range(NT)
        ]

        # ---- DMA loads, in first-use order ----
        # V first (V-projection is the PE prologue), then Q/K weights+acts,
        # then spatial/edge (needed by the first exp), then Wo (reuses the
        # wv slots once V-projection has consumed them).
        wv_t, xv_t = [], []
        for i in range(ND):
            t = wpool.tile([P, D], fp16, tag="wvo", name=f"wv{i}")
            nc.sync.dma_start(t, wv_d[i * P : (i + 1) * P, :])
            wv_t.append(t)
            t = xpool.tile([P, T], fp16, tag="xvs", name=f"xv{i}")
            nc.sync.dma_start(t, xvT_d[i * P : (i + 1) * P, :])
            xv_t.append(t)
        wq_t, xq_t = [], []
        for i in range(ND):
            t = wpool.tile([P, D], fp16, tag="wq", name=f"wq{i}")
            nc.sync.dma_start(t, wq_d[i * P : (i + 1) * P, :])
            wq_t.append(t)
            t = xpool.tile([P, T], fp16, tag="xq", name=f"xq{i}")
            nc.sync.dma_start(t, xqT_d[i * P : (i + 1) * P, :])
            xq_t.append(t)
        wk_t, xk_t = [], []
        for i in range(ND):
            t = wpool.tile([P, D], fp16, tag="wk", name=f"wk{i}")
            nc.sync.dma_start(t, wk_d[i * P : (i + 1) * P, :])
            wk_t.append(t)
            t = xpool.tile([P, T], fp16, tag="xk", name=f"xk{i}")
            nc.sync.dma_start(t, xkT_d[i * P : (i + 1) * P, :])
            xk_t.append(t)
        # spatial+edge -> expB^T (bf16: exp overflows fp16); reuses xv slots.
        for j in range(NT):
            sp_t = xpool.tile([P, T], fp16, tag="xvs", name=f"sp{j}")
            nc.sync.dma_start(sp_t, spT_d[j * P : (j + 1) * P, :])
            ed_t = xpool.tile([P, T], fp16, tag="xvs", name=f"ed{j}")
            nc.sync.dma_start(ed_t, edT_d[j * P : (j + 1) * P, :])
            bt = xpool.tile([P, T], fp16, tag="xvs", name=f"bt{j}")
            nc.vector.tensor_tensor(bt, sp_t, ed_t, AluOpType.add)
            nc.scalar.activation(expB_t[j], bt, AF.Exp)
        wo_t = []
        for i in range(ND):
            t = wpool.tile([P, D], fp16, tag="wvo", name=f"wo{i}")
            nc.sync.dma_start(t, wo_d[i * P : (i + 1) * P, :])
            wo_t.append(t)

        # ---- V projection (PE prologue; natural layout, head-interleaved) ----
        for tt in range(NT):
            vr = v_t[tt].rearrange("p (h c) -> p h c", c=DH + 1)
            psum = ps_pj.tile([P, T], fp32, tag="psP")
            for dt2 in range(ND):
                for ck in range(2):
                    cs = slice(ck * 512, (ck + 1) * 512)
                    nc.tensor.matmul(
                        psum[:, cs],
                        xv_t[dt2][:, tt * P : (tt + 1) * P],
                        wv_t[dt2][:, cs],
                        start=(dt2 == 0),
                        stop=(dt2 == ND - 1),
                    )
            dst = vr[:, :, 0:DH]
            src = psum.rearrange("p (h c) -> p h c", c=DH)
            if bv_nz:
                nc.vector.tensor_tensor(
                    dst,
                    src,
                    bvb_t.rearrange("p (h c) -> p h c", c=DH),
                    AluOpType.add,
                )
            else:
                nc.vector.tensor_copy(dst, src)
            nc.vector.memset(vr[:, :, DH : DH + 1], 1.0)

        # ---- Q/K projection emission helpers (chunked for interleaving) ----
        qk_psum = {}

        def proj_q_chunk(ft, dt2s):
            if dt2s[0] == 0:
                qk_psum["q", ft] = ps_pj.tile([P, T], fp32, tag="psP", name=f"pq{ft}")
            psum = qk_psum["q", ft]
            for dt2 in dt2s:
                for ck in range(2):
                    cs = slice(ck * 512, (ck + 1) * 512)
                    nc.tensor.matmul(
                        psum[:, cs],
                        wq_t[dt2][:, ft * P : (ft + 1) * P],
                        xq_t[dt2][:, cs],
                        start=(dt2 == 0),
                        stop=(dt2 == ND - 1),
                    )
            if dt2s[-1] == ND - 1:
                qT_t[ft] = epool.tile([P, T], fp16, tag="qT", bufs=3, name=f"qT{ft}")
                nc.vector.tensor_scalar(
                    qT_t[ft],
                    psum,
                    bq_t[:, ft : ft + 1],
                    float(SCALE),
                    AluOpType.add,
                    AluOpType.mult,
                )
                del qk_psum["q", ft]

        def proj_k_chunk(ft, dt2s):
            if dt2s[0] == 0:
                qk_psum["k", ft] = ps_pj.tile([P, T], fp32, tag="psP", name=f"pk{ft}")
            psum = qk_psum["k", ft]
            for dt2 in dt2s:
                for ck in range(2):
                    cs = slice(ck * 512, (ck + 1) * 512)
                    nc.tensor.matmul(
                        psum[:, cs],
                        wk_t[dt2][:, ft * P : (ft + 1) * P],
                        xk_t[dt2][:, cs],
                        start=(dt2 == 0),
                        stop=(dt2 == ND - 1),
                    )
            if dt2s[-1] == ND - 1:
                kT_t[ft] = epool.tile([P, T], fp16, tag="kT", bufs=3, name=f"kT{ft}")
                nc.vector.tensor_scalar_add(kT_t[ft], psum, bk_t[:, ft : ft + 1])
                del qk_psum["k", ft]

        all_dt2 = list(range(ND))
        proj_q_chunk(0, all_dt2)
        proj_k_chunk(0, all_dt2)

        # ---- attention steps ----
        e_tiles = {}

        def emit_scores(g, tkt):
            """Pair g's two heads, key-tile tkt: 4 matmuls, T0/T8 interleaved."""
            h0, h1 = 2 * g, 2 * g + 1
            sps0 = ps_sc.tile([P, T], fp32, tag="psS", name=f"s{h0}_{tkt}")
            sps1 = ps_sc.tile([P, T], fp32, tag="psS", name=f"s{h1}_{tkt}")
            for ck in range(2):
                cs = slice(ck * 512, (ck + 1) * 512)
                for hh, sps in ((0, sps0), (1, sps1)):
                    po = hh * DH
                    nc.tensor.matmul(
                        sps[:, cs],
                        kT_t[g][po : po + DH, tkt * P : (tkt + 1) * P],
                        qT_t[g][po : po + DH, cs],
                        start=True,
                        stop=True,
                    )
            for hh, sps in ((0, sps0), (1, sps1)):
                h = 2 * g + hh
                pt = epool.tile([P, T], bf16, tag="pt", bufs=2)
                nc.scalar.activation(
                    pt, sps, AF.Exp, bias=maskb_t[:, tkt : tkt + 1], scale=1.0
                )
                et = epool.tile([P, T], bf16, tag="et")
                nc.vector.tensor_tensor(et, pt, expB_t[tkt], AluOpType.mult)
                e_tiles[(h, tkt)] = et

        av_ps = {}

        def emit_av(h2, tkts):
            """AV accumulation for head h2 over the given key-tiles, in
            [65, 512] halves (65th PSUM row = softmax denominator)."""
            if tkts[0] == 0:
                av_ps[h2] = [
                    ps_av.tile([DH + 1, 512], fp32, tag="psA", name=f"av{h2}_{ck}")
                    for ck in range(2)
                ]
            avs = av_ps[h2]
            for tkt in tkts:
                vsl = v_t[tkt].rearrange("p (h c) -> p h c", c=DH + 1)[
                    :, h2 : h2 + 1, :
                ]
                et = e_tiles[(h2, tkt)]
                for ck in range(2):
                    cs = slice(ck * 512, (ck + 1) * 512)
                    nc.tensor.matmul(
                        avs[ck],
                        vsl,
                        et[:, cs],
                        start=(tkt == 0),
                        stop=(tkt == NT - 1),
                    )
                del e_tiles[(h2, tkt)]

        def emit_norm(h2):
            avs = av_ps.pop(h2)
            g2, po2 = h2 // 2, (h2 % 2) * DH
            for ck in range(2):
                cs = slice(ck * 512, (ck + 1) * 512)
                avp = avs[ck]
                den_sb = spool.tile([1, 512], fp32, tag="den", bufs=1)
                nc.vector.tensor_copy(den_sb, avp[DH : DH + 1, :])
                rde = spool.tile([1, 512], fp32, tag="rd", bufs=1)
                # NB: reciprocal_approx_fast misreads PSUM operands on HW
                nc.vector.reciprocal_approx_fast(rde, den_sb)
                rdb = spool.tile([DH, 512], fp32, tag="rdb")
                nc.gpsimd.partition_broadcast(rdb, rde)
                if po2 == 0:
                    nc.vector.tensor_tensor(
                        oT_t[g2][0:DH, cs], avp[0:DH, :], rdb, AluOpType.mult
                    )
                else:
                    stg = spool.tile([DH, 512], fp16, tag="stg")
                    nc.vector.tensor_tensor(stg, avp[0:DH, :], rdb, AluOpType.mult)
                    nc.sync.dma_start(oT_t[g2][DH:P, cs], stg)

        for g in range(NP):
            for j in range(NT):
                emit_scores(g, j)
                if g > 0:
                    h2 = 2 * (g - 1) + (0 if j < 4 else 1)
                    emit_av(h2, [2 * (j % 4), 2 * (j % 4) + 1])
                    if j == 3 or j == 7:
                        emit_norm(h2)
                if g + 1 < NP:
                    if j < 4:
                        proj_q_chunk(g + 1, [2 * j, 2 * j + 1])
                    else:
                        proj_k_chunk(g + 1, [2 * (j - 4), 2 * (j - 4) + 1])
        for j in range(NT):
            h2 = 2 * (NP - 1) + (0 if j < 4 else 1)
            emit_av(h2, [2 * (j % 4), 2 * (j % 4) + 1])
            if j == 3 or j == 7:
                emit_norm(h2)

        # ---- output projection: y[t, f] = OT.T @ Wo (+ bo) ----
        for tt in range(NT):
            ysb = ypool.tile([P, D], fp32, tag="yt")
            yps = ps_sc.tile([P, T], fp32, tag="psS")
            for g in range(ND):
                for ck in range(2):
                    cs = slice(ck * 512, (ck + 1) * 512)
                    nc.tensor.matmul(
                        yps[:, cs],
                        oT_t[g][:, tt * P : (tt + 1) * P],
                        wo_t[g][:, cs],
                        start=(g == 0),
                        stop=(g == ND - 1),
                    )
            if bo_nz:
                nc.vector.tensor_tensor(ysb, yps, bob_t, AluOpType.add)
            else:
                nc.vector.tensor_copy(ysb, yps)
            nc.sync.dma_start(y_d[tt * P : (tt + 1) * P, :], ysb)

    nc.compile()
    return nc


def make_in_maps(inputs):
    """Host-side shard + layout prep. Returns (in_maps, bv_nz, bo_nz)."""
    g = {k: np.asarray(v) for k, v in inputs.items()}
    f32 = np.float32

    wq = np.ascontiguousarray(g["Wq"].astype(FP16))
    wk = np.ascontiguousarray(g["Wk"].astype(FP16))
    wv = np.ascontiguousarray(g["Wv"].astype(FP16))
    wo = np.ascontiguousarray(g["Wo"].astype(FP16))
    bq2 = np.ascontiguousarray(g["bq"].astype(f32).reshape(ND, P).T)
    bk2 = np.ascontiguousarray(g["bk"].astype(f32).reshape(ND, P).T)
    bv = g["bv"].astype(f32)
    bo = g["bo"].astype(f32)
    bv_nz = bool(np.any(bv))
    bo_nz = bool(np.any(bo))

    in_maps = []
    for b in range(NCORES):
        m = {
            "xqT": np.ascontiguousarray(g["query"][b].T.astype(FP16)),
            "xkT": np.ascontiguousarray(g["key"][b].T.astype(FP16)),
            "xvT": np.ascontiguousarray(g["value"][b].T.astype(FP16)),
            "spT": np.ascontiguousarray(g["spatial_encoding"][b].T.astype(FP16)),
            "edT": np.ascontiguousarray(g["edge_encoding"][b].T.astype(FP16)),
            "wq": wq,
            "wk": wk,
            "wv": wv,
            "wo": wo,
            "maskneg": np.ascontiguousarray(
                np.where(g["key_padding_mask"][b], f32(NEG), f32(0.0))
                .astype(f32)
                .reshape(NT, P)
                .T
            ),
            "bq2": bq2,
            "bk2": bk2,
        }
        if bv_nz:
            m["bv1"] = bv.reshape(1, D)
        if bo_nz:
            m["bo1"] = bo.reshape(1, D)
        in_maps.append(m)
    return in_maps, bv_nz, bo_nz


def get_program(bv_nz, bo_nz):
    key = (bv_nz, bo_nz)
    if key not in _PROG_CACHE:
        _PROG_CACHE[key] = build_program(bv_nz, bo_nz)
    return _PROG_CACHE[key]


def kernel(**inputs) -> np.ndarray:
    from concourse.bass_utils import run_bass_kernel_spmd

    in_maps, bv_nz, bo_nz = make_in_maps(inputs)
    nc = get_program(bv_nz, bo_nz)
    res = run_bass_kernel_spmd(nc, in_maps, core_ids=list(range(NCORES)))
    out = np.stack([res.results[c]["y"] for c in range(NCORES)], axis=0)
    return out.astype(np.float32)


# revision 12
# speedup vs baseline: 1.5816x; 1.5816x over previous
"""Trainium2 Bass kernel for nn_MultiHeadGraphAttention.

Strategy: data-parallel over batch B=8 -> one batch element per NeuronCore,
zero collectives.  Per core, full 16-head attention over T=1024, D=1024.

v2: software-pipelined schedule.  The attention inner loop is ScalarE-bound
(16.8M exps ~ 110us at 1 elem/cycle/lane) while the projections are PE-bound
(~110us); v1 ran them in serial phases so each engine idled half the time.
v2 interleaves them:

  prologue:  V-projection (all 8 token tiles), expB = exp((sp+ed)^T),
             Q/K projection for head-pair group 0.
  step g=0..7, inner j=0..7:
     - scores for pair g, key-tile j: the two heads' K=64 matmuls sit at
       partition offsets 0/64 so bass auto-derives tile_position
       (0,0)/(64,0) and they run CONCURRENTLY in separate 64-row PE tiles;
       emission order T0,T8,T0,T8 maximizes the overlap.
     - exp + expB-multiply for both heads (ScalarE is the ~2us/j pacer).
     - AV for pair g-1: head h2a on j=0..3, h2b on j=4..7, two key-tiles
       per j, accumulating [65,512] halves (ones-column denominator trick).
     - 4 matmuls of next pair's Q (j<4) / K (j>=4) projection.
  epilogue:  AV(pair 7), output projection.

PSUM: scores 2x[128,1024] (4 banks) + AV 2x[65,512] (2) + proj [128,1024]
(2) = 8 banks exactly.

Other tricks unchanged from v1: fp16 activations/weights, exp(qk+bias) =
exp(qk)*expB with the key-padding mask riding ScalarE's per-partition bias
port, reciprocal_approx_fast on SBUF only, GpSimd partition-broadcast for
the denominator, odd heads staged through SBUF and DMA-copied into the
upper partition half of packed OT tiles.
"""

import os
import sys
from contextlib import ExitStack

import numpy as np

for _p in ("/opt/trn_rl_repo", "/root/.axon_site/_ro/trn_rl_repo"):
    if os.path.isdir(_p) and _p not in sys.path:
        sys.path.insert(0, _p)

import ml_dtypes

B, T, D, H = 8, 1024, 1024, 16
DH = D // H  # 64
P = 128
NT = T // P  # 8 token tiles
ND = D // P  # 8 feature tiles
NP = H // 2  # 8 head pairs
NCORES = 8
SCALE = 1.0 / np.sqrt(np.float32(DH))  # 0.125
NEG = -1.0e9
BF16 = ml_dtypes.bfloat16
FP16 = np.float16

_PROG_CACHE = {}


def build_program(
    bv_nz: bool = False,
    bo_nz: bool = False,
    loop_n: int = 0,
):
    import concourse.bass as bass
    import concourse.tile as tile
    from concourse import bacc, mybir
    from concourse.alu_op_type import AluOpType

    fp32 = mybir.dt.float32
    bf16 = mybir.dt.bfloat16
    fp16 = mybir.dt.float16
    AF = mybir.ActivationFunctionType

    nc = bacc.Bacc("TRN2", target_bir_lowering=False, debug=False)

    def din(name, shape, dt=fp16):
        return nc.dram_tensor(name, shape, dt, kind="ExternalInput").ap()

    xqT_d = din("xqT", [D, T])
    xkT_d = din("xkT", [D, T])
    xvT_d = din("xvT", [D, T])
    bT_d = din("bT", [T, T])
    wq_d = din("wq", [D, D])
    wk_d = din("wk", [D, D])
    wv_d = din("wv", [D, D])
    wo_d = din("wo", [D, D])
    maskneg_d = din("maskneg", [P, NT], fp32)
    bq_d = din("bq2", [P, ND], fp32)
    bk_d = din("bk2", [P, ND], fp32)
    bv_d = din("bv1", [1, D], fp32) if bv_nz else None
    bo_d = din("bo1", [1, D], fp32) if bo_nz else None
    y_d = nc.dram_tensor("y", [T, D], fp16, kind="ExternalOutput").ap()

    with tile.TileContext(nc) as tc, ExitStack() as ctx:
        pers = ctx.enter_context(tc.tile_pool(name="pers", bufs=1))
        xpool = ctx.enter_context(tc.tile_pool(name="xp", bufs=8))
        wpool = ctx.enter_context(tc.tile_pool(name="wp", bufs=8))
        epool = ctx.enter_context(tc.tile_pool(name="ep", bufs=13))
        ypool = ctx.enter_context(tc.tile_pool(name="yp", bufs=1))
        spool = ctx.enter_context(tc.tile_pool(name="sp", bufs=2))
        # PSUM budget (8 banks): scores 2x[128,1024]f32 = 4, AV 2x[65,512] = 2,
        # projections 1x[128,1024] = 2.
        ps_sc = ctx.enter_context(
            tc.tile_pool(name="psS", bufs=2, space=bass.MemorySpace.PSUM)
        )
        ps_av = ctx.enter_context(
            tc.tile_pool(name="psA", bufs=2, space=bass.MemorySpace.PSUM)
        )
        ps_pj = ctx.enter_context(
            tc.tile_pool(name="psP", bufs=1, space=bass.MemorySpace.PSUM)
        )
        if loop_n:
            ctx.enter_context(tc.For_i(0, loop_n, 1))

        # ---- small constants ----
        maskb_t = pers.tile([P, NT], fp32, tag="maskb")
        nc.sync.dma_start(maskb_t, maskneg_d)
        bq_t = pers.tile([P, ND], fp32, tag="bqt")
        nc.sync.dma_start(bq_t, bq_d)
        bk_t = pers.tile([P, ND], fp32, tag="bkt")
        nc.sync.dma_start(bk_t, bk_d)
        bvb_t = None
        if bv_nz:
            bv_row = pers.tile([1, D], fp32, tag="bvrow")
            nc.sync.dma_start(bv_row, bv_d)
            bvb_t = pers.tile([P, D], fp32, tag="bvb")
            nc.gpsimd.partition_broadcast(bvb_t, bv_row)
        bob_t = None
        if bo_nz:
            bo_row = pers.tile([1, D], fp32, tag="borow")
            nc.sync.dma_start(bo_row, bo_d)
            bob_t = pers.tile([P, D], fp32, tag="bob")
            nc.gpsimd.partition_broadcast(bob_t, bo_row)

        # ---- persistent activation tensors ----
        # qT/kT group tiles live only from their projection (step g-1) to
        # their scores (step g): ring of 3 instead of 8 persistent tiles.
        qT_t = {}
        kT_t = {}
        # V: per token-tile, heads interleaved in 65-column blocks (65th = 1.0)
        v_t = [pers.tile([P, H * (DH + 1)], bf16, tag=f"v{i}", name=f"v{i}") for i in range(NT)]
        oT_t = [pers.tile([P, T], fp16, tag=f"oT{i}", name=f"oT{i}") for i in range(ND)]
        expB_t = [
            pers.tile([P, T], bf16, tag=f"expB{j}", name=f"expB{j}") for j in range(NT)
        ]

        # ---- DMA loads, in first-use order ----
        # V first (V-projection is the PE prologue), then Q/K weights+acts,
        # then spatial/edge (needed by the first exp), then Wo (reuses the
        # wv slots once V-projection has consumed them).
        wv_t, xv_t = [], []
        for i in range(ND):
            t = wpool.tile([P, D], fp16, tag="wv", name=f"wv{i}")
            nc.sync.dma_start(t, wv_d[i * P : (i + 1) * P, :])
            wv_t.append(t)
            t = xpool.tile([P, T], fp16, tag="xvs", name=f"xv{i}")
            nc.sync.dma_start(t, xvT_d[i * P : (i + 1) * P, :])
            xv_t.append(t)
        wq_t, xq_t = [], []
        for i in range(ND):
            t = wpool.tile([P, D], fp16, tag="wq", name=f"wq{i}")
            nc.sync.dma_start(t, wq_d[i * P : (i + 1) * P, :])
            wq_t.append(t)
            t = xpool.tile([P, T], fp16, tag="xq", name=f"xq{i}")
            nc.sync.dma_start(t, xqT_d[i * P : (i + 1) * P, :])
            xq_t.append(t)
        wk_t, xk_t = [], []
        for i in range(ND):
            t = wpool.tile([P, D], fp16, tag="wk", name=f"wk{i}")
            nc.sync.dma_start(t, wk_d[i * P : (i + 1) * P, :])
            wk_t.append(t)
            t = xpool.tile([P, T], fp16, tag="xk", name=f"xk{i}")
            nc.sync.dma_start(t, xkT_d[i * P : (i + 1) * P, :])
            xk_t.append(t)
        # pre-added spatial+edge bias -> expB^T (bf16: exp overflows fp16);
        # reuses xv slots.
        for j in range(NT):
            bt = xpool.tile([P, T], fp16, tag="xvs", name=f"bt{j}")
            nc.sync.dma_start(bt, bT_d[j * P : (j + 1) * P, :])
            nc.scalar.activation(expB_t[j], bt, AF.Exp)
        wo_t = []
        for i in range(ND):
            t = wpool.tile([P, D], fp16, tag="wo", name=f"wo{i}")
            nc.sync.dma_start(t, wo_d[i * P : (i + 1) * P, :])
            wo_t.append(t)

        # ---- V projection (PE prologue; natural layout, head-interleaved) ----
        for tt in range(NT):
            vr = v_t[tt].rearrange("p (h c) -> p h c", c=DH + 1)
            psum = ps_pj.tile([P, T], fp32, tag="psP")
            for dt2 in range(ND):
                for ck in range(2):
                    cs = slice(ck * 512, (ck + 1) * 512)
                    nc.tensor.matmul(
                        psum[:, cs],
                        xv_t[dt2][:, tt * P : (tt + 1) * P],
                        wv_t[dt2][:, cs],
                        start=(dt2 == 0),
                        stop=(dt2 == ND - 1),
                    )
            dst = vr[:, :, 0:DH]
            src = psum.rearrange("p (h c) -> p h c", c=DH)
            if bv_nz:
                nc.vector.tensor_tensor(
                    dst,
                    src,
                    bvb_t.rearrange("p (h c) -> p h c", c=DH),
                    AluOpType.add,
                )
            else:
                nc.vector.tensor_copy(dst, src)
            nc.vector.memset(vr[:, :, DH : DH + 1], 1.0)

        # ---- Q/K projection emission helpers (chunked for interleaving) ----
        qk_psum = {}

        def proj_q_chunk(ft, dt2s):
            if dt2s[0] == 0:
                qk_psum["q", ft] = ps_pj.tile([P, T], fp32, tag="psP", name=f"pq{ft}")
            psum = qk_psum["q", ft]
            for dt2 in dt2s:
                for ck in range(2):
                    cs = slice(ck * 512, (ck + 1) * 512)
                    nc.tensor.matmul(
                        psum[:, cs],
                        wq_t[dt2][:, ft * P : (ft + 1) * P],
                        xq_t[dt2][:, cs],
                        start=(dt2 == 0),
                        stop=(dt2 == ND - 1),
                    )
            if dt2s[-1] == ND - 1:
                qT_t[ft] = epool.tile([P, T], fp16, tag="qT", bufs=2, name=f"qT{ft}")
                nc.vector.tensor_scalar(
                    qT_t[ft],
                    psum,
                    bq_t[:, ft : ft + 1],
                    float(SCALE),
                    AluOpType.add,
                    AluOpType.mult,
                )
                del qk_psum["q", ft]

        def proj_k_chunk(ft, dt2s):
            if dt2s[0] == 0:
                qk_psum["k", ft] = ps_pj.tile([P, T], fp32, tag="psP", name=f"pk{ft}")
            psum = qk_psum["k", ft]
            for dt2 in dt2s:
                for ck in range(2):
                    cs = slice(ck * 512, (ck + 1) * 512)
                    nc.tensor.matmul(
                        psum[:, cs],
                        wk_t[dt2][:, ft * P : (ft + 1) * P],
                        xk_t[dt2][:, cs],
                        start=(dt2 == 0),
                        stop=(dt2 == ND - 1),
                    )
            if dt2s[-1] == ND - 1:
                kT_t[ft] = epool.tile([P, T], fp16, tag="kT", bufs=2, name=f"kT{ft}")
                nc.vector.tensor_scalar_add(kT_t[ft], psum, bk_t[:, ft : ft + 1])
                del qk_psum["k", ft]

        all_dt2 = list(range(ND))
        proj_q_chunk(0, all_dt2)
        proj_k_chunk(0, all_dt2)

        # ---- attention steps ----
        e_tiles = {}

        def emit_scores(g, tkt):
            """Pair g's two heads, key-tile tkt: 4 matmuls, T0/T8 interleaved."""
            h0, h1 = 2 * g, 2 * g + 1
            sps0 = ps_sc.tile([P, T], fp32, tag="psS", name=f"s{h0}_{tkt}")
            sps1 = ps_sc.tile([P, T], fp32, tag="psS", name=f"s{h1}_{tkt}")
            for ck in range(2):
                cs = slice(ck * 512, (ck + 1) * 512)
                for hh, sps in ((0, sps0), (1, sps1)):
                    po = hh * DH
                    nc.tensor.matmul(
                        sps[:, cs],
                        kT_t[g][po : po + DH, tkt * P : (tkt + 1) * P],
                        qT_t[g][po : po + DH, cs],
                        start=True,
                        stop=True,
                    )
            for hh, sps in ((0, sps0), (1, sps1)):
                h = 2 * g + hh
                pt = epool.tile([P, T], bf16, tag="pt", bufs=2)
                nc.scalar.activation(
                    pt, sps, AF.Exp, bias=maskb_t[:, tkt : tkt + 1], scale=1.0
                )
                et = epool.tile([P, T], bf16, tag="et")
                nc.vector.tensor_tensor(et, pt, expB_t[tkt], AluOpType.mult)
                e_tiles[(h, tkt)] = et

        av_ps = {}

        def emit_av(h2, tkts):
            """AV accumulation for head h2 over the given key-tiles, in
            [65, 512] halves (65th PSUM row = softmax denominator)."""
            if tkts[0] == 0:
                av_ps[h2] = [
                    ps_av.tile([DH + 1, 512], fp32, tag="psA", name=f"av{h2}_{ck}")
                    for ck in range(2)
                ]
            avs = av_ps[h2]
            for tkt in tkts:
                vsl = v_t[tkt].rearrange("p (h c) -> p h c", c=DH + 1)[
                    :, h2 : h2 + 1, :
                ]
                et = e_tiles[(h2, tkt)]
                for ck in range(2):
                    cs = slice(ck * 512, (ck + 1) * 512)
                    nc.tensor.matmul(
                        avs[ck],
                        vsl,
                        et[:, cs],
                        start=(tkt == 0),
                        stop=(tkt == NT - 1),
                    )
                del e_tiles[(h2, tkt)]

        def emit_norm(h2):
            avs = av_ps.pop(h2)
            g2, po2 = h2 // 2, (h2 % 2) * DH
            for ck in range(2):
                cs = slice(ck * 512, (ck + 1) * 512)
                avp = avs[ck]
                den_sb = spool.tile([1, 512], fp32, tag="den", bufs=1)
                nc.vector.tensor_copy(den_sb, avp[DH : DH + 1, :])
                rde = spool.tile([1, 512], fp32, tag="rd", bufs=1)
                # NB: reciprocal_approx_fast misreads PSUM operands on HW
                nc.vector.reciprocal_approx_fast(rde, den_sb)
                rdb = spool.tile([DH, 512], fp32, tag="rdb", bufs=1)
                nc.gpsimd.partition_broadcast(rdb, rde)
                if po2 == 0:
                    nc.vector.tensor_tensor(
                        oT_t[g2][0:DH, cs], avp[0:DH, :], rdb, AluOpType.mult
                    )
                else:
                    stg = spool.tile([DH, 512], fp16, tag="stg", bufs=1)
                    nc.vector.tensor_tensor(stg, avp[0:DH, :], rdb, AluOpType.mult)
                    nc.sync.dma_start(oT_t[g2][DH:P, cs], stg)

        def av_unit(k):
            h2, m = k // 4, k % 4
            emit_av(h2, [2 * m, 2 * m + 1])
            if m == 3:
                emit_norm(h2)

        LAG = 5
        slot = 0
        for g in range(NP):
            for j in range(NT):
                emit_scores(g, j)
                if 0 <= slot - LAG < 4 * H:
                    av_unit(slot - LAG)
                if g + 1 < NP:
                    if j < 4:
                        proj_q_chunk(g + 1, [2 * j, 2 * j + 1])
                    else:
                        proj_k_chunk(g + 1, [2 * (j - 4), 2 * (j - 4) + 1])
                slot += 1
        for k in range(slot - LAG, 4 * H):
            av_unit(k)

        # ---- output projection: y[t, f] = OT.T @ Wo (+ bo) ----
        for tt in range(NT):
            ysb = ypool.tile([P, D], fp16, tag="yt")
            yps = ps_sc.tile([P, T], fp32, tag="psS")
            for g in range(ND):
                for ck in range(2):
                    cs = slice(ck * 512, (ck + 1) * 512)
                    nc.tensor.matmul(
                        yps[:, cs],
                        oT_t[g][:, tt * P : (tt + 1) * P],
                        wo_t[g][:, cs],
                        start=(g == 0),
                        stop=(g == ND - 1),
                    )
            if bo_nz:
                nc.vector.tensor_tensor(ysb, yps, bob_t, AluOpType.add)
            else:
                nc.vector.tensor_copy(ysb, yps)
            nc.sync.dma_start(y_d[tt * P : (tt + 1) * P, :], ysb)

    nc.compile()
    return nc


def make_in_maps(inputs):
    """Host-side shard + layout prep. Returns (in_maps, bv_nz, bo_nz)."""
    g = {k: np.asarray(v) for k, v in inputs.items()}
    f32 = np.float32

    wq = np.ascontiguousarray(g["Wq"].astype(FP16))
    wk = np.ascontiguousarray(g["Wk"].astype(FP16))
    wv = np.ascontiguousarray(g["Wv"].astype(FP16))
    wo = np.ascontiguousarray(g["Wo"].astype(FP16))
    bq2 = np.ascontiguousarray(g["bq"].astype(f32).reshape(ND, P).T)
    bk2 = np.ascontiguousarray(g["bk"].astype(f32).reshape(ND, P).T)
    bv = g["bv"].astype(f32)
    bo = g["bo"].astype(f32)
    bv_nz = bool(np.any(bv))
    bo_nz = bool(np.any(bo))

    in_maps = []
    for b in range(NCORES):
        m = {
            "xqT": np.ascontiguousarray(g["query"][b].T.astype(FP16)),
            "xkT": np.ascontiguousarray(g["key"][b].T.astype(FP16)),
            "xvT": np.ascontiguousarray(g["value"][b].T.astype(FP16)),
            "bT": np.ascontiguousarray(
                (g["spatial_encoding"][b] + g["edge_encoding"][b]).T.astype(FP16)
            ),
            "wq": wq,
            "wk": wk,
            "wv": wv,
            "wo": wo,
            "maskneg": np.ascontiguousarray(
                np.where(g["key_padding_mask"][b], f32(NEG), f32(0.0))
                .astype(f32)
                .reshape(NT, P)
                .T
            ),
            "bq2": bq2,
            "bk2": bk2,
        }
        if bv_nz:
            m["bv1"] = bv.reshape(1, D)
        if bo_nz:
            m["bo1"] = bo.reshape(1, D)
        in_maps.append(m)
    return in_maps, bv_nz, bo_nz


def get_program(bv_nz, bo_nz):
    key = (bv_nz, bo_nz)
    if key not in _PROG_CACHE:
        _PROG_CACHE[key] = build_program(bv_nz, bo_nz)
    return _PROG_CACHE[key]


def kernel(**inputs) -> np.ndarray:
    from concourse.bass_utils import run_bass_kernel_spmd

    in_maps, bv_nz, bo_nz = make_in_maps(inputs)
    nc = get_program(bv_nz, bo_nz)
    res = run_bass_kernel_spmd(nc, in_maps, core_ids=list(range(NCORES)))
    out = np.stack([res.results[c]["y"] for c in range(NCORES)], axis=0)
    return out.astype(np.float32)
